# revision 14
# baseline (speedup 1.0000x reference)
"""Trainium2 Bass kernel for a 3-layer GNN message-passing block.

Reference computation (per layer i):
    x1 = h @ Wfc[i] + bfc[i]                        # [N_SUB, D]
    x2 = scatter_mean(h, idx) @ Wsum[i] + bsum[i]   # [NUM_GRAPHS, D]
    h  = elu(x1 + x2[idx])
then
    out = relu(scatter_mean(h, idx) @ Wf1 + bf1) @ Wf2 + bf2

Strategy: data-parallel over 8 NeuronCores; the sorted graph index gives each
core 5 contiguous windows of <=128 graphs / 2560 padded rows. scatter/gather
are one-hot matmuls on TensorE. Row-major h; layer-1 fc runs in fp16 from a
host-pretransposed copy; layers 2/3 fc run as fp8 DoubleRowSWInterleave
matmuls whose stationary operand comes from a 2-byte DMA transpose of fp8
feature PAIRS (bitcast u16) - the byte-interleaved output is exactly the
SWInterleave weight layout. SWInterleave reverses the output partition
(row) order, which is handled by per-layer-parity one-hot variants
(wgaA/wgaB, wscA/wscB) packed host-side. Scatters run as fp8 DoubleRow
matmuls over chunk pairs directly on the row-major fp8 h tiles. The fp8
weight-quantization residual dW = Wfc - q8(Wfc) is folded into Wsum
host-side (x1 and x2 add into the same pre-activation; h rows share a large
per-graph common mode that mean @ dW recovers), keeping rel err ~1.5e-2.
ELU uses h = max(z, min(exp(z),1)-1): exp on ScalarE, min on GpSimd, max on
VectorE.
"""

import numpy as np

P = 128
D = 512
N_SUB = 100000
NUM_GRAPHS = 4096
N_LAYERS = 3
NUM_TASKS = 10
N_CORES = 8
WIN_PER_CORE = 5
CH_PER_WIN = 20
ROWS_PER_WIN = CH_PER_WIN * P            # 2560
N_LOC = WIN_PER_CORE * ROWS_PER_WIN      # 12800 padded rows per core
CHUNKS = N_LOC // P                      # 100
N_PAIRS = CHUNKS // 2                    # 50
G_WIN = P                                # graph slots per window
G_LOC = WIN_PER_CORE * G_WIN             # 640 graph slots per core
N_WIN_TOTAL = N_CORES * WIN_PER_CORE     # 40
DBLK = D // P                            # 4
D2 = 2 * D                               # 1024
D2BLK = D2 // P
PREFETCH = 5     # next-layer chunk-pair transposes prefetched

_cached = {}


def _f16():
    return np.float16


def _f8():
    import ml_dtypes
    return ml_dtypes.float8_e4m3fn


def _q8(x):
    return np.asarray(x, np.float32).astype(_f8()).astype(np.float32)


# ----------------------------------------------------------------- host prep

def _pack_windows(counts):
    total = int(counts.sum())
    target = total / N_WIN_TOTAL
    wins = []
    g = 0
    rows_done = 0
    for w in range(N_WIN_TOTAL):
        g0 = g
        rows_w = 0
        while g < NUM_GRAPHS:
            c = int(counts[g])
            if rows_w + c > ROWS_PER_WIN or (g - g0) >= G_WIN:
                break
            if (w < N_WIN_TOTAL - 1 and rows_w > 0
                    and rows_done + rows_w + c > (w + 1) * target):
                remaining = total - (rows_done + rows_w)
                if remaining <= (N_WIN_TOTAL - w - 1) * ROWS_PER_WIN * 0.98:
                    break
            rows_w += c
            g += 1
        while g < NUM_GRAPHS and counts[g] == 0 and (g - g0) < G_WIN:
            g += 1
        rows_done += rows_w
        wins.append((g0, g))
    assert g == NUM_GRAPHS, f"window packing failed: {g}/{NUM_GRAPHS}"
    return wins


def _build_core_inputs(h, idx, counts, starts, wins, core, shared):
    f16 = _f16()
    f8 = _f8()
    h_pad = np.zeros((N_LOC, D), dtype=np.float32)
    slot = np.full(N_LOC, -1, dtype=np.int64)
    invc = np.zeros((P, WIN_PER_CORE), dtype=np.float32)
    gmap = []
    for lw in range(WIN_PER_CORE):
        g0, g1 = wins[core * WIN_PER_CORE + lw]
        r0, r1 = int(starts[g0]), int(starts[g1])
        n = r1 - r0
        h_pad[lw * ROWS_PER_WIN: lw * ROWS_PER_WIN + n] = h[r0:r1]
        slot[lw * ROWS_PER_WIN: lw * ROWS_PER_WIN + n] = \
            lw * G_WIN + (idx[r0:r1] - g0)
        for j, g in enumerate(range(g0, g1)):
            invc[j, lw] = 1.0 / max(int(counts[g]), 1)
            gmap.append((g, lw * G_WIN + j))
    # per-chunk one-hots; B variants have the row order reversed within each
    # chunk (SWInterleave fc reverses psum/h row order on odd fc layers)
    wsc = np.zeros((CHUNKS, P, P), dtype=np.float32)   # [c][row, g]
    for c in range(CHUNKS):
        w = c // CH_PER_WIN
        s = slot[c * P:(c + 1) * P]
        real = np.nonzero(s >= 0)[0]
        wsc[c][real, s[real] - w * G_WIN] = 1.0
    wsc_rev = wsc[:, ::-1, :]
    wga = np.transpose(wsc, (0, 2, 1))                 # [c][g, row]
    wga_rev = np.transpose(wsc_rev, (0, 2, 1))

    def flat(a, dt):
        return np.ascontiguousarray(
            np.transpose(a, (1, 0, 2)).reshape(P, CHUNKS * P)).astype(dt)

    # scatter pair layout: [p, pc, i, g] = wsc[2pc+i][p, g]
    def scpair(a):
        return np.ascontiguousarray(
            a.reshape(N_PAIRS, 2, P, P).transpose(2, 0, 1, 3)
            .reshape(P, N_PAIRS * 2 * P)).astype(f8)

    h3 = h_pad.reshape(CHUNKS, P, DBLK, P)
    h16t = np.ascontiguousarray(h3.transpose(3, 0, 2, 1).reshape(P, CHUNKS * D))
    Wsum0, cbias0 = shared["_wsum0"], shared["_cbias0"]
    ssum = np.zeros((G_LOC, D), dtype=np.float32)
    np.add.at(ssum, slot[slot >= 0], h_pad[slot >= 0])
    mean0 = ssum * invc.T.reshape(G_LOC, 1)
    x2w0 = (mean0 @ Wsum0 + cbias0).astype(f16)
    x2w0b = np.ascontiguousarray(
        x2w0.reshape(WIN_PER_CORE, G_WIN, D).transpose(1, 0, 2)
        .reshape(G_WIN, WIN_PER_CORE * D))
    in_map = {
        "h16t": h16t.astype(f16),
        "x2w0b": x2w0b,
        "wscA": scpair(wsc),
        "wscB": scpair(wsc_rev),
        "wgaA": flat(wga, f16),
        "wgaB": flat(wga_rev, f16),
        "invc": invc,
        **{k: v for k, v in shared.items() if not k.startswith("_")},
    }
    return in_map, gmap


def _prep_shared(Wfc, bfc, Wsum, bsum, Wf1, bf1, Wf2, bf2):
    f16 = _f16()
    f8 = _f8()
    # layer-0 fc weights fp16 (moving): [P, b, D] = Wfc[0][128b+p, :]
    wfc0 = np.ascontiguousarray(
        Wfc[0].reshape(DBLK, P, D).transpose(1, 0, 2)
        .reshape(P, DBLK * D)).astype(f16)
    # layers 1/2 fc weights fp8 DoubleRow moving pairs:
    # [p, li, b, i, :] = Wfc[l][256b + 2p + i, :]
    wfc8 = np.zeros((P, 2, 2, 2, D), dtype=np.float32)
    for li, l in enumerate((1, 2)):
        Wl = Wfc[l].astype(np.float32)
        for b in range(2):
            for i in range(2):
                wfc8[:, li, b, i, :] = Wl[256 * b + 2 * np.arange(P) + i]
    wfc8 = np.ascontiguousarray(wfc8.reshape(P, 2 * 2 * 2 * D)).astype(f8)
    Wsum_eff = Wsum.astype(np.float32).copy()
    for l in (1, 2):
        Wsum_eff[l] += Wfc[l].astype(np.float32) - _q8(Wfc[l])
    wsum = np.ascontiguousarray(
        Wsum_eff.reshape(N_LAYERS, DBLK, P, D).transpose(2, 0, 1, 3)
        .reshape(P, N_LAYERS * DBLK * D)).astype(f16)
    cbias = np.ascontiguousarray((bfc + bsum).reshape(1, N_LAYERS * D)).astype(f16)
    wf1 = np.ascontiguousarray(
        Wf1.reshape(DBLK, P, 2, D).transpose(1, 0, 2, 3)
        .reshape(P, DBLK * D2)).astype(f16)
    bf1w = np.ascontiguousarray(bf1.reshape(1, D2)).astype(f16)
    wf2 = np.ascontiguousarray(
        Wf2.reshape(D2BLK, P, NUM_TASKS).transpose(1, 0, 2)
        .reshape(P, D2BLK * NUM_TASKS)).astype(f16)
    bf2w = np.ascontiguousarray(bf2.reshape(1, NUM_TASKS)).astype(f16)
    return {
        "wfc0": wfc0, "wfc8": wfc8, "wsum": wsum, "cbias": cbias,
        "wf1": wf1, "bf1w": bf1w, "wf2": wf2, "bf2w": bf2w,
        "_wsum0": Wsum[0].astype(np.float32),
        "_cbias0": (bfc[0] + bsum[0]).astype(np.float32)[None, :],
    }


# -------------------------------------------------------------- bass program

def _build_program():
    from contextlib import ExitStack

    import concourse.mybir as mybir
    import concourse.tile as tile
    from concourse import bacc

    bf = mybir.dt.float16
    f8 = mybir.dt.float8e4
    f32 = mybir.dt.float32
    AF = mybir.ActivationFunctionType
    ALU = mybir.AluOpType
    DR = mybir.MatmulPerfMode.DoubleRow
    DRS = mybir.MatmulPerfMode.DoubleRowSwInterleave

    nc = bacc.Bacc("TRN2", debug=False, target_bir_lowering=False,
                   num_devices=N_CORES, dynamic_dma_scratch_size=2048)

    h16t_d = nc.dram_tensor("h16t", [P, CHUNKS * D], bf, kind="ExternalInput")
    x2w0_d = nc.dram_tensor("x2w0b", [G_WIN, WIN_PER_CORE * D], bf,
                            kind="ExternalInput")
    wscA_d = nc.dram_tensor("wscA", [P, N_PAIRS * 2 * P], f8, kind="ExternalInput")
    wscB_d = nc.dram_tensor("wscB", [P, N_PAIRS * 2 * P], f8, kind="ExternalInput")
    wgaA_d = nc.dram_tensor("wgaA", [P, CHUNKS * P], bf, kind="ExternalInput")
    wgaB_d = nc.dram_tensor("wgaB", [P, CHUNKS * P], bf, kind="ExternalInput")
    invc_d = nc.dram_tensor("invc", [P, WIN_PER_CORE], f32, kind="ExternalInput")
    wfc0_d = nc.dram_tensor("wfc0", [P, DBLK * D], bf, kind="ExternalInput")
    wfc8_d = nc.dram_tensor("wfc8", [P, 2 * 2 * 2 * D], f8, kind="ExternalInput")
    wsum_d = nc.dram_tensor("wsum", [P, N_LAYERS * DBLK * D], bf, kind="ExternalInput")
    cbias_d = nc.dram_tensor("cbias", [1, N_LAYERS * D], bf, kind="ExternalInput")
    wf1_d = nc.dram_tensor("wf1", [P, DBLK * D2BLK * P], bf, kind="ExternalInput")
    bf1_d = nc.dram_tensor("bf1w", [1, D2], bf, kind="ExternalInput")
    wf2_d = nc.dram_tensor("wf2", [P, D2BLK * NUM_TASKS], bf, kind="ExternalInput")
    bf2_d = nc.dram_tensor("bf2w", [1, NUM_TASKS], bf, kind="ExternalInput")
    out_d = nc.dram_tensor("out", [NUM_TASKS, G_LOC], f32, kind="ExternalOutput")

    with tile.TileContext(nc) as tc, ExitStack() as ctx:
        const = ctx.enter_context(tc.tile_pool(name="const", bufs=1))
        hpool = ctx.enter_context(tc.tile_pool(name="h", bufs=1))
        stream = ctx.enter_context(tc.tile_pool(name="stream", bufs=6))
        work = ctx.enter_context(tc.tile_pool(name="work", bufs=2))
        x2pool = ctx.enter_context(tc.tile_pool(name="x2", bufs=1))
        psum = ctx.enter_context(tc.tile_pool(name="psum", bufs=1, space="PSUM"))
        psx1 = ctx.enter_context(tc.tile_pool(name="psx1", bufs=6, space="PSUM"))

        ones = const.tile([1, P], bf, tag="ones")
        nc.vector.memset(ones[:], 1.0)
        x2w0_t = []
        for w in range(WIN_PER_CORE):
            t0w = x2pool.tile([P, D], bf, tag=f"x2w{w}", name=f"x2w0_{w}")
            nc.sync.dma_start(t0w[:], x2w0_d[:, w * D:(w + 1) * D])
            x2w0_t.append(t0w)
        WCH = CH_PER_WIN * P
        wgaA_w = [const.tile([P, WCH], bf, tag=f"wgaA{k}", name=f"wgaA{k}")
                  for k in range(WIN_PER_CORE)]
        wgaB_w = [const.tile([P, WCH], bf, tag=f"wgaB{k}", name=f"wgaB{k}")
                  for k in range(WIN_PER_CORE)]
        wscA_w = [const.tile([P, WCH], f8, tag=f"wscA{k}", name=f"wscA{k}")
                  for k in range(WIN_PER_CORE)]
        wscB_w = [const.tile([P, WCH], f8, tag=f"wscB{k}", name=f"wscB{k}")
                  for k in range(WIN_PER_CORE)]
        wfc0_t = const.tile([P, DBLK * D], bf, tag="wfc0")
        nc.sync.dma_start(wfc0_t[:], wfc0_d[:, :])
        nc.sync.dma_start(wgaA_w[0][:], wgaA_d[:, 0 * WCH:1 * WCH])
        invc_t = const.tile([P, WIN_PER_CORE], f32, tag="invc")
        nc.sync.dma_start(invc_t[:], invc_d[:, :])
        # fp8 row-major h, chunk-pair tiles [128, 2, 512]
        h_t = [hpool.tile([P, 2 * D], f8, tag=f"h{pc}", name=f"h{pc}")
               for pc in range(N_PAIRS)]

        def h_chunk(c):
            return h_t[c // 2][:, (c % 2) * D:(c % 2 + 1) * D]

        hT0 = []

        def load_hT0(c):
            ht = stream.tile([P, D], bf, tag="hT", name=f"hT0_{c}", bufs=12)
            nc.sync.dma_start(ht[:], h16t_d[:, c * D:(c + 1) * D])
            hT0.append(ht)

        for c in range(6):
            load_hT0(c)
        for k in range(WIN_PER_CORE):
            nc.sync.dma_start(wscA_w[k][:], wscA_d[:, k * WCH:(k + 1) * WCH])
            if k + 1 < WIN_PER_CORE:
                nc.sync.dma_start(wgaA_w[k + 1][:],
                                  wgaA_d[:, (k + 1) * WCH:(k + 2) * WCH])
            for c in range(6 + k * 6, 6 + (k + 1) * 6):
                load_hT0(c)
        for c in range(36, CHUNKS):
            load_hT0(c)
        # second-parity one-hots + remaining weights on the ACT HWDGE queue
        wfc8_t = const.tile([P, 2 * 2 * 2 * D], f8, tag="wfc8")
        nc.scalar.dma_start(wfc8_t[:], wfc8_d[:, :])
        for k in range(WIN_PER_CORE):
            nc.scalar.dma_start(wgaB_w[k][:], wgaB_d[:, k * WCH:(k + 1) * WCH])
            nc.scalar.dma_start(wscB_w[k][:], wscB_d[:, k * WCH:(k + 1) * WCH])
        wsumb = const.tile([P, N_LAYERS * DBLK * D], bf, tag="wsumb")
        nc.scalar.dma_start(wsumb[:], wsum_d[:, :])
        cbiasb = const.tile([1, N_LAYERS * D], bf, tag="cbiasb")
        nc.scalar.dma_start(cbiasb[:], cbias_d[:, :])
        wf1b = const.tile([P, DBLK * D2BLK * P], bf, tag="wf1b")
        nc.scalar.dma_start(wf1b[:], wf1_d[:, :])
        bf1_t = const.tile([1, D2], bf, tag="bf1")
        nc.scalar.dma_start(bf1_t[:], bf1_d[:, :])
        wf2b = const.tile([P, D2BLK * NUM_TASKS], bf, tag="wf2b")
        nc.scalar.dma_start(wf2b[:], wf2_d[:, :])
        bf2_t = const.tile([1, NUM_TASKS], bf, tag="bf2")
        nc.scalar.dma_start(bf2_t[:], bf2_d[:, :])

        def wfc0_s(b):
            return wfc0_t[:, b * D:(b + 1) * D]

        def wfc8_s(layer, b):
            off = (((layer - 1) * 2 + b) * 2) * D
            return wfc8_t[:, off:off + 2 * D].rearrange("p (i d) -> p i d", i=2)

        def wsum_s(layer, b):
            return wsumb[:, (layer * DBLK + b) * D:(layer * DBLK + b + 1) * D]

        def cbias_s(layer):
            return cbiasb[:, layer * D:(layer + 1) * D]

        def wf1_h(b, half):
            i = b * 2 + half
            return wf1b[:, i * D:(i + 1) * D]

        def wf2_s(q):
            return wf2b[:, q * NUM_TASKS:(q + 1) * NUM_TASKS]

        # parity: h order is natural after layers 0/2, reversed after layer 1
        def wga_c(c, layer):
            tiles = wgaB_w if layer == 1 else wgaA_w
            return tiles[c // CH_PER_WIN][:, (c % CH_PER_WIN) * P:
                                          (c % CH_PER_WIN + 1) * P]

        def wsc_pair(pc, layer):
            # scatter of h produced by `layer`: reversed iff layer == 1
            tiles = wscB_w if layer == 1 else wscA_w
            j = (2 * pc) % CH_PER_WIN
            return tiles[(2 * pc) // CH_PER_WIN][:, j * P:(j + 2) * P] \
                .rearrange("p (i g) -> p i g", i=2)

        hT_next = hT0

        def transpose_pair(pc, tag, name):
            """chunk-pair fp8 -> SWInterleave lhsT via u16-pair DMA transpose."""
            ht = stream.tile([P, 2 * D], f8, tag=tag, name=name, bufs=8)
            nc.sync.dma_start(
                ht[:].bitcast(bf).rearrange("p (k r) -> p k r", k=4),
                h_t[pc][:].bitcast(bf), transpose=True)
            return ht

        def x2_window(meanT, w, layer):
            ps = psum.tile([P, D], f32, tag="x2")
            for b in range(DBLK):
                nc.tensor.matmul(ps[:], lhsT=meanT[:, b * P:(b + 1) * P],
                                 rhs=wsum_s(layer, b),
                                 start=(b == 0), stop=False)
            nc.tensor.matmul(ps[:], lhsT=ones[:, :P], rhs=cbias_s(layer),
                             start=False, stop=True)
            x2w = x2pool.tile([P, D], bf, tag=f"x2w{w}", name=f"x2w{layer}_{w}")
            nc.scalar.activation(x2w[:], ps[:], AF.Copy)
            return x2w[:]

        x2ws = {w: x2w0_t[w][:] for w in range(WIN_PER_CORE)}

        out_sb = const.tile([NUM_TASKS, G_LOC], f32, tag="out")

        head_pend = {}

        def head_t(w, meanT):
            t = work.tile([P, D2], bf, tag="tT", bufs=1, name=f"t{w}")
            tTh = []
            for half in range(2):
                ps = psx1.tile([P, D], f32, tag="x1", name=f"hps{w}_{half}")
                for b in range(DBLK):
                    nc.tensor.matmul(ps[:], lhsT=meanT[:, b * P:(b + 1) * P],
                                     rhs=wf1_h(b, half),
                                     start=(b == 0), stop=False)
                nc.tensor.matmul(ps[:], lhsT=ones[:, :P],
                                 rhs=bf1_t[:, half * D:(half + 1) * D],
                                 start=False, stop=True)
                nc.scalar.activation(t[:, half * D:(half + 1) * D],
                                     ps[:], AF.Relu)
                th = work.tile([P, D], bf, tag=f"tTh{half}", bufs=2,
                               name=f"tTh{w}_{half}")
                nc.scalar.dma_start(
                    th[:].rearrange("p (b r) -> p b r", b=DBLK),
                    t[:, half * D:(half + 1) * D], transpose=True)
                tTh.append(th)
            head_pend[w] = tTh

        def head_out(w):
            tTh = head_pend.pop(w)
            pso = psum.tile([NUM_TASKS, P], f32, tag="x2", name=f"pso{w}")
            for q in range(D2BLK):
                nc.tensor.matmul(pso[:], lhsT=wf2_s(q),
                                 rhs=tTh[q // 4][:, (q % 4) * P:
                                                 (q % 4 + 1) * P],
                                 start=(q == 0), stop=False)
            nc.tensor.matmul(pso[:], lhsT=bf2_t[:], rhs=ones[:, :P],
                             start=False, stop=True)
            nc.vector.tensor_copy(out_sb[:, w * P:(w + 1) * P], pso[:])

        LAG = 4
        for layer in range(N_LAYERS):
            hTts = hT_next
            if layer > 0:
                for pc in range(len(hTts), N_PAIRS):
                    hTts.append(
                        transpose_pair(pc, "hT8", f"hT{layer}_{pc}"))
            hT_next = []
            nxt_x2ws = {}
            sc_state = {}
            pend = {}

            def emit_pair_scatter(pc, layer=layer, sc_state=sc_state,
                                  pend=pend):
                i = pc % (CH_PER_WIN // 2)
                w = pc // (CH_PER_WIN // 2)
                if i == 0:
                    sc_state["ps"] = psum.tile([P, D], f32, tag="sc",
                                               name=f"sc{layer}_{w}")
                nc.tensor.matmul(
                    sc_state["ps"][:], lhsT=wsc_pair(pc, layer),
                    rhs=h_t[pc][:].rearrange("p (i d) -> p i d", i=2),
                    start=(i == 0), stop=(i == CH_PER_WIN // 2 - 1),
                    perf_mode=DR)
                if i == CH_PER_WIN // 2 - 1:
                    mean = work.tile([P, D], bf, tag="mean", bufs=1,
                                     name=f"mean{layer}_{w}")
                    nc.vector.tensor_scalar(mean[:], sc_state["ps"][:],
                                            invc_t[:, w:w + 1], None,
                                            op0=ALU.mult)
                    meanT = work.tile([P, D], bf, tag="meanT", bufs=1,
                                      name=f"meanT{layer}_{w}")
                    nc.scalar.dma_start(
                        meanT[:].rearrange("p (b r) -> p b r", b=DBLK),
                        mean[:], transpose=True)
                    pend[w] = meanT

            def emit_window_tail(w, layer=layer, nxt_x2ws=nxt_x2ws,
                                 pend=pend):
                meanT = pend.pop(w)
                if layer < N_LAYERS - 1:
                    nxt_x2ws[w] = x2_window(meanT, w, layer + 1)
                else:
                    head_t(w, meanT)

            for c in range(CHUNKS):
                w = c // CH_PER_WIN
                ps = psx1.tile([P, D], f32, tag="x1")
                if layer == 0:
                    hTt = hTts[c]
                    for b in range(DBLK):
                        nc.tensor.matmul(ps[:], lhsT=hTt[:, b * P:(b + 1) * P],
                                         rhs=wfc0_s(b),
                                         start=(b == 0), stop=False)
                else:
                    hTt = hTts[c // 2]
                    for b in range(2):
                        lhsT = hTt[:, (2 * (c % 2) + b) * 2 * P:
                                   (2 * (c % 2) + b + 1) * 2 * P] \
                            .rearrange("p (m i) -> p m i", i=2)
                        nc.tensor.matmul(ps[:], lhsT=lhsT,
                                         rhs=wfc8_s(layer, b),
                                         start=(b == 0), stop=False,
                                         perf_mode=DRS)
                nc.tensor.matmul(ps[:], lhsT=wga_c(c, layer), rhs=x2ws[w],
                                 start=False, stop=True)
                # ELU: h = max(z, min(exp(z), 1) - 1)
                e = work.tile([P, D], bf, tag="e", bufs=3)
                nc.scalar.activation(e[:], ps[:], AF.Exp)
                me = work.tile([P, D], bf, tag="me", bufs=3)
                nc.vector.tensor_scalar(me[:], e[:], 1.0, -1.0,
                                        op0=ALU.min, op1=ALU.add)
                nc.vector.tensor_tensor(h_chunk(c), ps[:], me[:],
                                        op=ALU.max)
                if layer < N_LAYERS - 1 and c % 2 == 1 and c < 2 * PREFETCH:
                    hT_next.append(
                        transpose_pair(c // 2, "hT8p", f"hTp{layer}_{c // 2}"))
                if c >= LAG and (c - LAG) % 2 == 1:
                    emit_pair_scatter((c - LAG) // 2)
                cw = (c - LAG - 8) // CH_PER_WIN
                if c >= LAG + 8 and (c - LAG - 8) % CH_PER_WIN == CH_PER_WIN - 1:
                    emit_window_tail(cw)
                cw2 = (c - LAG - 14) // CH_PER_WIN
                if (c >= LAG + 14
                        and (c - LAG - 14) % CH_PER_WIN == CH_PER_WIN - 1
                        and cw2 in head_pend):
                    head_out(cw2)
            for pc in range((CHUNKS - LAG) // 2, N_PAIRS):
                emit_pair_scatter(pc)
            for w in sorted(pend):
                emit_window_tail(w)
            for w in sorted(head_pend):
                head_out(w)
            x2ws = nxt_x2ws

        nc.sync.dma_start(out_d[:, :], out_sb[:])

    nc.compile()
    return nc


# ------------------------------------------------------------------- kernel

def kernel(**inputs):
    h = np.asarray(inputs["h_subgraph"], dtype=np.float32)
    idx = np.asarray(inputs["subgraph_idx_batch"]).astype(np.int64)
    if not np.all(idx[:-1] <= idx[1:]):
        order = np.argsort(idx, kind="stable")
        h, idx = h[order], idx[order]

    counts = np.bincount(idx, minlength=NUM_GRAPHS)
    starts = np.concatenate([[0], np.cumsum(counts)])
    wins = _pack_windows(counts)
    shared = _prep_shared(
        np.asarray(inputs["Wfc"], np.float32), np.asarray(inputs["bfc"], np.float32),
        np.asarray(inputs["Wsum"], np.float32), np.asarray(inputs["bsum"], np.float32),
        np.asarray(inputs["Wf1"], np.float32), np.asarray(inputs["bf1"], np.float32),
        np.asarray(inputs["Wf2"], np.float32), np.asarray(inputs["bf2"], np.float32),
    )

    in_maps = []
    gmaps = []
    for core in range(N_CORES):
        m, gm = _build_core_inputs(h, idx, counts, starts, wins, core, shared)
        in_maps.append(m)
        gmaps.append(gm)

    _cached["in_maps"] = in_maps
    if "nc" not in _cached:
        _cached["nc"] = _build_program()
    nc = _cached["nc"]

    from concourse import bass_utils
    res = bass_utils.run_bass_kernel_spmd(
        nc, in_maps, core_ids=list(range(N_CORES)))

    out = np.zeros((NUM_GRAPHS, NUM_TASKS), dtype=np.float32)
    for core in range(N_CORES):
        o = res.results[core]["out"]           # [10, 640]
        for g, s in gmaps[core]:
            out[g] = o[:, s]
    return out


# revision 18
# speedup vs baseline: 1.0861x; 1.0861x over previous
"""Trainium2 Bass kernel for a 3-layer GNN message-passing block.

Reference computation (per layer i):
    x1 = h @ Wfc[i] + bfc[i]                        # [N_SUB, D]
    x2 = scatter_mean(h, idx) @ Wsum[i] + bsum[i]   # [NUM_GRAPHS, D]
    h  = elu(x1 + x2[idx])
then
    out = relu(scatter_mean(h, idx) @ Wf1 + bf1) @ Wf2 + bf2

Strategy: data-parallel over 8 NeuronCores; the sorted graph index gives each
core 5 contiguous windows of <=128 graphs / 2560 padded rows. scatter/gather
are one-hot matmuls on TensorE. Row-major h; layer-1 fc runs in fp16 from a
host-pretransposed copy; layers 2/3 fc run as fp8 DoubleRowSWInterleave
matmuls whose stationary operand comes from a 2-byte DMA transpose of fp8
feature PAIRS (bitcast u16) - the byte-interleaved output is exactly the
SWInterleave weight layout. SWInterleave reverses the output partition
(row) order, which is handled by per-layer-parity one-hot variants
(wgaA/wgaB, wscA/wscB) packed host-side. Scatters run as fp8 DoubleRow
matmuls over chunk pairs directly on the row-major fp8 h tiles. The fp8
weight-quantization residual dW = Wfc - q8(Wfc) is folded into Wsum
host-side (x1 and x2 add into the same pre-activation; h rows share a large
per-graph common mode that mean @ dW recovers), keeping rel err ~1.5e-2.
ELU uses h = max(z, min(exp(z),1)-1): exp on ScalarE, min on GpSimd, max on
VectorE.
"""

import numpy as np

P = 128
D = 512
N_SUB = 100000
NUM_GRAPHS = 4096
N_LAYERS = 3
NUM_TASKS = 10
N_CORES = 8
WIN_PER_CORE = 5
CH_PER_WIN = 20
ROWS_PER_WIN = CH_PER_WIN * P            # 2560
N_LOC = WIN_PER_CORE * ROWS_PER_WIN      # 12800 padded rows per core
CHUNKS = N_LOC // P                      # 100
N_PAIRS = CHUNKS // 2                    # 50
G_WIN = P                                # graph slots per window
G_LOC = WIN_PER_CORE * G_WIN             # 640 graph slots per core
N_WIN_TOTAL = N_CORES * WIN_PER_CORE     # 40
DBLK = D // P                            # 4
D2 = 2 * D                               # 1024
D2BLK = D2 // P
PREFETCH = 5     # next-layer chunk-pair transposes prefetched

_cached = {}


def _f16():
    return np.float16


def _f8():
    import ml_dtypes
    return ml_dtypes.float8_e4m3fn


def _q8(x):
    return np.asarray(x, np.float32).astype(_f8()).astype(np.float32)


# ----------------------------------------------------------------- host prep

def _pack_windows(counts):
    total = int(counts.sum())
    target = total / N_WIN_TOTAL
    wins = []
    g = 0
    rows_done = 0
    for w in range(N_WIN_TOTAL):
        g0 = g
        rows_w = 0
        while g < NUM_GRAPHS:
            c = int(counts[g])
            if rows_w + c > ROWS_PER_WIN or (g - g0) >= G_WIN:
                break
            if (w < N_WIN_TOTAL - 1 and rows_w > 0
                    and rows_done + rows_w + c > (w + 1) * target):
                remaining = total - (rows_done + rows_w)
                if remaining <= (N_WIN_TOTAL - w - 1) * ROWS_PER_WIN * 0.98:
                    break
            rows_w += c
            g += 1
        while g < NUM_GRAPHS and counts[g] == 0 and (g - g0) < G_WIN:
            g += 1
        rows_done += rows_w
        wins.append((g0, g))
    assert g == NUM_GRAPHS, f"window packing failed: {g}/{NUM_GRAPHS}"
    return wins


def _build_core_inputs(h, idx, counts, starts, wins, core, shared):
    f16 = _f16()
    f8 = _f8()
    h_pad = np.zeros((N_LOC, D), dtype=np.float32)
    slot = np.full(N_LOC, -1, dtype=np.int64)
    invc = np.zeros((P, WIN_PER_CORE), dtype=np.float32)
    gmap = []
    for lw in range(WIN_PER_CORE):
        g0, g1 = wins[core * WIN_PER_CORE + lw]
        r0, r1 = int(starts[g0]), int(starts[g1])
        n = r1 - r0
        h_pad[lw * ROWS_PER_WIN: lw * ROWS_PER_WIN + n] = h[r0:r1]
        slot[lw * ROWS_PER_WIN: lw * ROWS_PER_WIN + n] = \
            lw * G_WIN + (idx[r0:r1] - g0)
        for j, g in enumerate(range(g0, g1)):
            invc[j, lw] = 1.0 / max(int(counts[g]), 1)
            gmap.append((g, lw * G_WIN + j))
    # per-chunk one-hots; B variants have the row order reversed within each
    # chunk (SWInterleave fc reverses psum/h row order on odd fc layers)
    wsc = np.zeros((CHUNKS, P, P), dtype=np.float32)   # [c][row, g]
    for c in range(CHUNKS):
        w = c // CH_PER_WIN
        s = slot[c * P:(c + 1) * P]
        real = np.nonzero(s >= 0)[0]
        wsc[c][real, s[real] - w * G_WIN] = 1.0
    wsc_rev = wsc[:, ::-1, :]
    wga = np.transpose(wsc, (0, 2, 1))                 # [c][g, row]
    wga_rev = np.transpose(wsc_rev, (0, 2, 1))

    def flat(a, dt):
        return np.ascontiguousarray(
            np.transpose(a, (1, 0, 2)).reshape(P, CHUNKS * P)).astype(dt)

    # scatter pair layout: [p, pc, i, g] = wsc[2pc+i][p, g]
    def scpair(a):
        return np.ascontiguousarray(
            a.reshape(N_PAIRS, 2, P, P).transpose(2, 0, 1, 3)
            .reshape(P, N_PAIRS * 2 * P)).astype(f8)

    h3 = h_pad.reshape(CHUNKS, P, DBLK, P)
    h16t = np.ascontiguousarray(h3.transpose(3, 0, 2, 1).reshape(P, CHUNKS * D))
    Wsum0, cbias0 = shared["_wsum0"], shared["_cbias0"]
    ssum = np.zeros((G_LOC, D), dtype=np.float32)
    np.add.at(ssum, slot[slot >= 0], h_pad[slot >= 0])
    mean0 = ssum * invc.T.reshape(G_LOC, 1)
    x2w0 = (mean0 @ Wsum0 + cbias0).astype(f16)
    x2w0b = np.ascontiguousarray(
        x2w0.reshape(WIN_PER_CORE, G_WIN, D).transpose(1, 0, 2)
        .reshape(G_WIN, WIN_PER_CORE * D))
    in_map = {
        "h16t": h16t.astype(f16),
        "x2w0b": x2w0b,
        "wscA": scpair(wsc),
        "wscB": scpair(wsc_rev),
        "wgaA": flat(wga, f16),
        "wgaB": flat(wga_rev, f16),
        "invc": invc,
        **{k: v for k, v in shared.items() if not k.startswith("_")},
    }
    return in_map, gmap


def _prep_shared(Wfc, bfc, Wsum, bsum, Wf1, bf1, Wf2, bf2):
    f16 = _f16()
    f8 = _f8()
    # layer-0 fc weights fp16 (moving): [P, b, D] = Wfc[0][128b+p, :]
    wfc0 = np.ascontiguousarray(
        Wfc[0].reshape(DBLK, P, D).transpose(1, 0, 2)
        .reshape(P, DBLK * D)).astype(f16)
    # layers 1/2 fc weights fp8 DoubleRow moving pairs:
    # [p, li, b, i, :] = Wfc[l][256b + 2p + i, :]
    wfc8 = np.zeros((P, 2, 2, 2, D), dtype=np.float32)
    for li, l in enumerate((1, 2)):
        Wl = Wfc[l].astype(np.float32)
        for b in range(2):
            for i in range(2):
                wfc8[:, li, b, i, :] = Wl[256 * b + 2 * np.arange(P) + i]
    wfc8 = np.ascontiguousarray(wfc8.reshape(P, 2 * 2 * 2 * D)).astype(f8)
    Wsum_eff = Wsum.astype(np.float32).copy()
    for l in (1, 2):
        Wsum_eff[l] += Wfc[l].astype(np.float32) - _q8(Wfc[l])
    wsum = np.ascontiguousarray(
        Wsum_eff.reshape(N_LAYERS, DBLK, P, D).transpose(2, 0, 1, 3)
        .reshape(P, N_LAYERS * DBLK * D)).astype(f16)
    cbias = np.ascontiguousarray((bfc + bsum).reshape(1, N_LAYERS * D)).astype(f16)
    wf1 = np.ascontiguousarray(
        Wf1.reshape(DBLK, P, 2, D).transpose(1, 0, 2, 3)
        .reshape(P, DBLK * D2)).astype(f16)
    bf1w = np.ascontiguousarray(bf1.reshape(1, D2)).astype(f16)
    wf2 = np.ascontiguousarray(
        Wf2.reshape(D2BLK, P, NUM_TASKS).transpose(1, 0, 2)
        .reshape(P, D2BLK * NUM_TASKS)).astype(f16)
    bf2w = np.ascontiguousarray(bf2.reshape(1, NUM_TASKS)).astype(f16)
    return {
        "wfc0": wfc0, "wfc8": wfc8, "wsum": wsum, "cbias": cbias,
        "wf1": wf1, "bf1w": bf1w, "wf2": wf2, "bf2w": bf2w,
        "_wsum0": Wsum[0].astype(np.float32),
        "_cbias0": (bfc[0] + bsum[0]).astype(np.float32)[None, :],
    }


# -------------------------------------------------------------- bass program

def _build_program():
    from contextlib import ExitStack

    import concourse.mybir as mybir
    import concourse.tile as tile
    from concourse import bacc

    bf = mybir.dt.float16
    f8 = mybir.dt.float8e4
    f32 = mybir.dt.float32
    AF = mybir.ActivationFunctionType
    ALU = mybir.AluOpType
    DR = mybir.MatmulPerfMode.DoubleRow
    DRS = mybir.MatmulPerfMode.DoubleRowSwInterleave

    nc = bacc.Bacc("TRN2", debug=False, target_bir_lowering=False,
                   num_devices=N_CORES, dynamic_dma_scratch_size=2048)

    h16t_d = nc.dram_tensor("h16t", [P, CHUNKS * D], bf, kind="ExternalInput")
    x2w0_d = nc.dram_tensor("x2w0b", [G_WIN, WIN_PER_CORE * D], bf,
                            kind="ExternalInput")
    wscA_d = nc.dram_tensor("wscA", [P, N_PAIRS * 2 * P], f8, kind="ExternalInput")
    wscB_d = nc.dram_tensor("wscB", [P, N_PAIRS * 2 * P], f8, kind="ExternalInput")
    wgaA_d = nc.dram_tensor("wgaA", [P, CHUNKS * P], bf, kind="ExternalInput")
    wgaB_d = nc.dram_tensor("wgaB", [P, CHUNKS * P], bf, kind="ExternalInput")
    invc_d = nc.dram_tensor("invc", [P, WIN_PER_CORE], f32, kind="ExternalInput")
    wfc0_d = nc.dram_tensor("wfc0", [P, DBLK * D], bf, kind="ExternalInput")
    wfc8_d = nc.dram_tensor("wfc8", [P, 2 * 2 * 2 * D], f8, kind="ExternalInput")
    wsum_d = nc.dram_tensor("wsum", [P, N_LAYERS * DBLK * D], bf, kind="ExternalInput")
    cbias_d = nc.dram_tensor("cbias", [1, N_LAYERS * D], bf, kind="ExternalInput")
    wf1_d = nc.dram_tensor("wf1", [P, DBLK * D2BLK * P], bf, kind="ExternalInput")
    bf1_d = nc.dram_tensor("bf1w", [1, D2], bf, kind="ExternalInput")
    wf2_d = nc.dram_tensor("wf2", [P, D2BLK * NUM_TASKS], bf, kind="ExternalInput")
    bf2_d = nc.dram_tensor("bf2w", [1, NUM_TASKS], bf, kind="ExternalInput")
    out_d = nc.dram_tensor("out", [NUM_TASKS, G_LOC], f32, kind="ExternalOutput")

    with tile.TileContext(nc) as tc, ExitStack() as ctx:
        const = ctx.enter_context(tc.tile_pool(name="const", bufs=1))
        hpool = ctx.enter_context(tc.tile_pool(name="h", bufs=1))
        stream = ctx.enter_context(tc.tile_pool(name="stream", bufs=6))
        work = ctx.enter_context(tc.tile_pool(name="work", bufs=2))
        x2pool = ctx.enter_context(tc.tile_pool(name="x2", bufs=1))
        psum = ctx.enter_context(tc.tile_pool(name="psum", bufs=1, space="PSUM"))
        psx1 = ctx.enter_context(tc.tile_pool(name="psx1", bufs=6, space="PSUM"))

        ones = const.tile([1, P], bf, tag="ones")
        nc.vector.memset(ones[:], 1.0)
        x2w0_t = []
        for w in range(WIN_PER_CORE):
            t0w = x2pool.tile([P, D], bf, tag=f"x2w{w}", name=f"x2w0_{w}")
            nc.sync.dma_start(t0w[:], x2w0_d[:, w * D:(w + 1) * D])
            x2w0_t.append(t0w)
        WCH = CH_PER_WIN * P
        wgaA_w = [const.tile([P, WCH], bf, tag=f"wgaA{k}", name=f"wgaA{k}")
                  for k in range(WIN_PER_CORE)]
        wgaB_w = [const.tile([P, WCH], bf, tag=f"wgaB{k}", name=f"wgaB{k}")
                  for k in range(WIN_PER_CORE)]
        wscA_w = [const.tile([P, WCH], f8, tag=f"wscA{k}", name=f"wscA{k}")
                  for k in range(WIN_PER_CORE)]
        wscB_w = [const.tile([P, WCH], f8, tag=f"wscB{k}", name=f"wscB{k}")
                  for k in range(WIN_PER_CORE)]
        wfc0_t = const.tile([P, DBLK * D], bf, tag="wfc0")
        nc.sync.dma_start(wfc0_t[:], wfc0_d[:, :])
        invc_t = const.tile([P, WIN_PER_CORE], f32, tag="invc")
        nc.sync.dma_start(invc_t[:], invc_d[:, :])
        # fp8 row-major h, chunk-pair tiles [128, 2, 512]
        h_t = [hpool.tile([P, 2 * D], f8, tag=f"h{pc}", name=f"h{pc}")
               for pc in range(N_PAIRS)]

        def h_chunk(c):
            return h_t[c // 2][:, (c % 2) * D:(c % 2 + 1) * D]

        # h0T streamed in 4-chunk tiles (few large DMAs keep the sync queue
        # short so the layer-1/2 transposes are not stuck behind it)
        hT0 = []
        for q in range(CHUNKS // 4):
            ht = stream.tile([P, 4 * D], bf, tag="hT", name=f"hT0_{q}", bufs=3)
            nc.sync.dma_start(ht[:], h16t_d[:, q * 4 * D:(q + 1) * 4 * D])
            hT0.append(ht)
        # one-hots + weights on the ACT HWDGE queue (A-parity first: needed
        # from the very first layer-0 chunk)
        for k in range(WIN_PER_CORE):
            nc.scalar.dma_start(wgaA_w[k][:], wgaA_d[:, k * WCH:(k + 1) * WCH])
            nc.scalar.dma_start(wscA_w[k][:], wscA_d[:, k * WCH:(k + 1) * WCH])
        wfc8_t = const.tile([P, 2 * 2 * 2 * D], f8, tag="wfc8")
        nc.scalar.dma_start(wfc8_t[:], wfc8_d[:, :])
        for k in range(WIN_PER_CORE):
            nc.scalar.dma_start(wgaB_w[k][:], wgaB_d[:, k * WCH:(k + 1) * WCH])
            nc.scalar.dma_start(wscB_w[k][:], wscB_d[:, k * WCH:(k + 1) * WCH])
        wsumb = const.tile([P, N_LAYERS * DBLK * D], bf, tag="wsumb")
        nc.scalar.dma_start(wsumb[:], wsum_d[:, :])
        cbiasb = const.tile([1, N_LAYERS * D], bf, tag="cbiasb")
        nc.scalar.dma_start(cbiasb[:], cbias_d[:, :])
        wf1b = const.tile([P, DBLK * D2BLK * P], bf, tag="wf1b")
        nc.scalar.dma_start(wf1b[:], wf1_d[:, :])
        bf1_t = const.tile([1, D2], bf, tag="bf1")
        nc.scalar.dma_start(bf1_t[:], bf1_d[:, :])
        wf2b = const.tile([P, D2BLK * NUM_TASKS], bf, tag="wf2b")
        nc.scalar.dma_start(wf2b[:], wf2_d[:, :])
        bf2_t = const.tile([1, NUM_TASKS], bf, tag="bf2")
        nc.scalar.dma_start(bf2_t[:], bf2_d[:, :])

        def wfc0_s(b):
            return wfc0_t[:, b * D:(b + 1) * D]

        def wfc8_s(layer, b):
            off = (((layer - 1) * 2 + b) * 2) * D
            return wfc8_t[:, off:off + 2 * D].rearrange("p (i d) -> p i d", i=2)

        def wsum_s(layer, b):
            return wsumb[:, (layer * DBLK + b) * D:(layer * DBLK + b + 1) * D]

        def cbias_s(layer):
            return cbiasb[:, layer * D:(layer + 1) * D]

        def wf1_h(b, half):
            i = b * 2 + half
            return wf1b[:, i * D:(i + 1) * D]

        def wf2_s(q):
            return wf2b[:, q * NUM_TASKS:(q + 1) * NUM_TASKS]

        # parity: h order is natural after layers 0/2, reversed after layer 1
        def wga_c(c, layer):
            tiles = wgaB_w if layer == 1 else wgaA_w
            return tiles[c // CH_PER_WIN][:, (c % CH_PER_WIN) * P:
                                          (c % CH_PER_WIN + 1) * P]

        def wsc_pair(pc, layer):
            # scatter of h produced by `layer`: reversed iff layer == 1
            tiles = wscB_w if layer == 1 else wscA_w
            j = (2 * pc) % CH_PER_WIN
            return tiles[(2 * pc) // CH_PER_WIN][:, j * P:(j + 2) * P] \
                .rearrange("p (i g) -> p i g", i=2)

        hT_next = hT0

        def transpose_pair(pc, tag, name, eng=None):
            """chunk-pair fp8 -> SWInterleave lhsT via u16-pair DMA transpose."""
            ht = stream.tile([P, 2 * D], f8, tag=tag, name=name, bufs=12)
            (eng or nc.sync).dma_start(
                ht[:].bitcast(bf).rearrange("p (k r) -> p k r", k=4),
                h_t[pc][:].bitcast(bf), transpose=True)
            return ht

        def x2_window(meanT, w, layer):
            ps = psum.tile([P, D], f32, tag="x2")
            for b in range(DBLK):
                nc.tensor.matmul(ps[:], lhsT=meanT[:, b * P:(b + 1) * P],
                                 rhs=wsum_s(layer, b),
                                 start=(b == 0), stop=False)
            nc.tensor.matmul(ps[:], lhsT=ones[:, :P], rhs=cbias_s(layer),
                             start=False, stop=True)
            x2w = x2pool.tile([P, D], bf, tag=f"x2w{w}", name=f"x2w{layer}_{w}")
            nc.scalar.activation(x2w[:], ps[:], AF.Copy)
            return x2w[:]

        x2ws = {w: x2w0_t[w][:] for w in range(WIN_PER_CORE)}

        out_sb = const.tile([NUM_TASKS, G_LOC], f32, tag="out")

        head_pend = {}

        def head_t(w, meanT):
            t = work.tile([P, D2], bf, tag="tT", bufs=1, name=f"t{w}")
            tTh = []
            for half in range(2):
                ps = psx1.tile([P, D], f32, tag="x1", name=f"hps{w}_{half}")
                for b in range(DBLK):
                    nc.tensor.matmul(ps[:], lhsT=meanT[:, b * P:(b + 1) * P],
                                     rhs=wf1_h(b, half),
                                     start=(b == 0), stop=False)
                nc.tensor.matmul(ps[:], lhsT=ones[:, :P],
                                 rhs=bf1_t[:, half * D:(half + 1) * D],
                                 start=False, stop=True)
                nc.scalar.activation(t[:, half * D:(half + 1) * D],
                                     ps[:], AF.Relu)
                th = work.tile([P, D], bf, tag=f"tTh{half}", bufs=2,
                               name=f"tTh{w}_{half}")
                nc.scalar.dma_start(
                    th[:].rearrange("p (b r) -> p b r", b=DBLK),
                    t[:, half * D:(half + 1) * D], transpose=True)
                tTh.append(th)
            head_pend[w] = tTh

        def head_out(w):
            tTh = head_pend.pop(w)
            pso = psum.tile([NUM_TASKS, P], f32, tag="x2", name=f"pso{w}")
            for q in range(D2BLK):
                nc.tensor.matmul(pso[:], lhsT=wf2_s(q),
                                 rhs=tTh[q // 4][:, (q % 4) * P:
                                                 (q % 4 + 1) * P],
                                 start=(q == 0), stop=False)
            nc.tensor.matmul(pso[:], lhsT=bf2_t[:], rhs=ones[:, :P],
                             start=False, stop=True)
            nc.vector.tensor_copy(out_sb[:, w * P:(w + 1) * P], pso[:])

        LAG = 4
        for layer in range(N_LAYERS):
            hTts = hT_next
            if layer > 0:
                for pc in range(len(hTts), N_PAIRS):
                    hTts.append(
                        transpose_pair(pc, "hT8", f"hT{layer}_{pc}"))
            hT_next = []
            nxt_x2ws = {}
            sc_state = {}
            pend = {}

            def emit_pair_scatter(pc, layer=layer, sc_state=sc_state,
                                  pend=pend):
                i = pc % (CH_PER_WIN // 2)
                w = pc // (CH_PER_WIN // 2)
                if i == 0:
                    sc_state["ps"] = psum.tile([P, D], f32, tag="sc",
                                               name=f"sc{layer}_{w}")
                nc.tensor.matmul(
                    sc_state["ps"][:], lhsT=wsc_pair(pc, layer),
                    rhs=h_t[pc][:].rearrange("p (i d) -> p i d", i=2),
                    start=(i == 0), stop=(i == CH_PER_WIN // 2 - 1),
                    perf_mode=DR)
                if i == CH_PER_WIN // 2 - 1:
                    mean = work.tile([P, D], bf, tag="mean", bufs=1,
                                     name=f"mean{layer}_{w}")
                    nc.vector.tensor_scalar(mean[:], sc_state["ps"][:],
                                            invc_t[:, w:w + 1], None,
                                            op0=ALU.mult)
                    meanT = work.tile([P, D], bf, tag="meanT", bufs=1,
                                      name=f"meanT{layer}_{w}")
                    nc.scalar.dma_start(
                        meanT[:].rearrange("p (b r) -> p b r", b=DBLK),
                        mean[:], transpose=True)
                    pend[w] = meanT

            def emit_window_tail(w, layer=layer, nxt_x2ws=nxt_x2ws,
                                 pend=pend):
                meanT = pend.pop(w)
                if layer < N_LAYERS - 1:
                    nxt_x2ws[w] = x2_window(meanT, w, layer + 1)
                else:
                    head_t(w, meanT)

            for c in range(CHUNKS):
                w = c // CH_PER_WIN
                ps = psx1.tile([P, D], f32, tag="x1")
                if layer == 0:
                    hTt = hTts[c // 4]
                    co = (c % 4) * D
                    for b in range(DBLK):
                        nc.tensor.matmul(ps[:],
                                         lhsT=hTt[:, co + b * P:
                                                  co + (b + 1) * P],
                                         rhs=wfc0_s(b),
                                         start=(b == 0), stop=False)
                else:
                    hTt = hTts[c // 2]
                    for b in range(2):
                        lhsT = hTt[:, (2 * (c % 2) + b) * 2 * P:
                                   (2 * (c % 2) + b + 1) * 2 * P] \
                            .rearrange("p (m i) -> p m i", i=2)
                        nc.tensor.matmul(ps[:], lhsT=lhsT,
                                         rhs=wfc8_s(layer, b),
                                         start=(b == 0), stop=False,
                                         perf_mode=DRS)
                nc.tensor.matmul(ps[:], lhsT=wga_c(c, layer), rhs=x2ws[w],
                                 start=False, stop=True)
                # ELU: h = max(z, min(exp(z), 1) - 1)
                e = work.tile([P, D], bf, tag="e", bufs=3)
                nc.scalar.activation(e[:], ps[:], AF.Exp)
                me = work.tile([P, D], bf, tag="me", bufs=3)
                nc.vector.tensor_scalar(me[:], e[:], 1.0, -1.0,
                                        op0=ALU.min, op1=ALU.add)
                nc.vector.tensor_tensor(h_chunk(c), ps[:], me[:],
                                        op=ALU.max)
                if layer < N_LAYERS - 1 and c % 2 == 1 and c < 2 * PREFETCH:
                    hT_next.append(
                        transpose_pair(c // 2, "hT8p", f"hTp{layer}_{c // 2}",
                                       eng=nc.scalar))
                if c >= LAG and (c - LAG) % 2 == 1:
                    emit_pair_scatter((c - LAG) // 2)
                cw = (c - LAG - 8) // CH_PER_WIN
                if c >= LAG + 8 and (c - LAG - 8) % CH_PER_WIN == CH_PER_WIN - 1:
                    emit_window_tail(cw)
                cw2 = (c - LAG - 14) // CH_PER_WIN
                if (c >= LAG + 14
                        and (c - LAG - 14) % CH_PER_WIN == CH_PER_WIN - 1
                        and cw2 in head_pend):
                    head_out(cw2)
            for pc in range((CHUNKS - LAG) // 2, N_PAIRS):
                emit_pair_scatter(pc)
            for w in sorted(pend):
                emit_window_tail(w)
            for w in sorted(head_pend):
                head_out(w)
            x2ws = nxt_x2ws

        nc.sync.dma_start(out_d[:, :], out_sb[:])

    nc.compile()
    return nc


# ------------------------------------------------------------------- kernel

def kernel(**inputs):
    h = np.asarray(inputs["h_subgraph"], dtype=np.float32)
    idx = np.asarray(inputs["subgraph_idx_batch"]).astype(np.int64)
    if not np.all(idx[:-1] <= idx[1:]):
        order = np.argsort(idx, kind="stable")
        h, idx = h[order], idx[order]

    counts = np.bincount(idx, minlength=NUM_GRAPHS)
    starts = np.concatenate([[0], np.cumsum(counts)])
    wins = _pack_windows(counts)
    shared = _prep_shared(
        np.asarray(inputs["Wfc"], np.float32), np.asarray(inputs["bfc"], np.float32),
        np.asarray(inputs["Wsum"], np.float32), np.asarray(inputs["bsum"], np.float32),
        np.asarray(inputs["Wf1"], np.float32), np.asarray(inputs["bf1"], np.float32),
        np.asarray(inputs["Wf2"], np.float32), np.asarray(inputs["bf2"], np.float32),
    )

    in_maps = []
    gmaps = []
    for core in range(N_CORES):
        m, gm = _build_core_inputs(h, idx, counts, starts, wins, core, shared)
        in_maps.append(m)
        gmaps.append(gm)

    _cached["in_maps"] = in_maps
    if "nc" not in _cached:
        _cached["nc"] = _build_program()
    nc = _cached["nc"]

    from concourse import bass_utils
    res = bass_utils.run_bass_kernel_spmd(
        nc, in_maps, core_ids=list(range(N_CORES)))

    out = np.zeros((NUM_GRAPHS, NUM_TASKS), dtype=np.float32)
    for core in range(N_CORES):
        o = res.results[core]["out"]           # [10, 640]
        for g, s in gmaps[core]:
            out[g] = o[:, s]
    return out


# revision 32
# speedup vs baseline: 1.0866x; 1.0005x over previous
"""Trainium2 Bass kernel for a 3-layer GNN message-passing block.

Reference computation (per layer i):
    x1 = h @ Wfc[i] + bfc[i]                        # [N_SUB, D]
    x2 = scatter_mean(h, idx) @ Wsum[i] + bsum[i]   # [NUM_GRAPHS, D]
    h  = elu(x1 + x2[idx])
then
    out = relu(scatter_mean(h, idx) @ Wf1 + bf1) @ Wf2 + bf2

Strategy: data-parallel over 8 NeuronCores; the sorted graph index gives each
core 5 contiguous windows of <=128 graphs / 2560 padded rows. scatter/gather
are one-hot matmuls on TensorE. Row-major h; layer-1 fc runs in fp16 from a
host-pretransposed copy; layers 2/3 fc run as fp8 DoubleRowSWInterleave
matmuls whose stationary operand comes from a 2-byte DMA transpose of fp8
feature PAIRS (bitcast u16) - the byte-interleaved output is exactly the
SWInterleave weight layout. SWInterleave reverses the output partition
(row) order, which is handled by per-layer-parity one-hot variants
(wgaA/wgaB, wscA/wscB) packed host-side. Scatters run as fp8 DoubleRow
matmuls over chunk pairs directly on the row-major fp8 h tiles. The fp8
weight-quantization residual dW = Wfc - q8(Wfc) is folded into Wsum
host-side (x1 and x2 add into the same pre-activation; h rows share a large
per-graph common mode that mean @ dW recovers), keeping rel err ~1.5e-2.
ELU uses h = max(z, min(exp(z),1)-1): exp on ScalarE, min on GpSimd, max on
VectorE.
"""

import numpy as np

P = 128
D = 512
N_SUB = 100000
NUM_GRAPHS = 4096
N_LAYERS = 3
NUM_TASKS = 10
N_CORES = 8
WIN_PER_CORE = 5
CH_PER_WIN = 20
ROWS_PER_WIN = CH_PER_WIN * P            # 2560
N_LOC = WIN_PER_CORE * ROWS_PER_WIN      # 12800 padded rows per core
CHUNKS = N_LOC // P                      # 100
N_PAIRS = CHUNKS // 2                    # 50
G_WIN = P                                # graph slots per window
G_LOC = WIN_PER_CORE * G_WIN             # 640 graph slots per core
N_WIN_TOTAL = N_CORES * WIN_PER_CORE     # 40
DBLK = D // P                            # 4
D2 = 2 * D                               # 1024
D2BLK = D2 // P
PREFETCH = 5     # next-layer chunk-pair transposes prefetched

_cached = {}


def _f16():
    return np.float16


def _f8():
    import ml_dtypes
    return ml_dtypes.float8_e4m3fn


def _q8(x):
    return np.asarray(x, np.float32).astype(_f8()).astype(np.float32)


# ----------------------------------------------------------------- host prep

def _pack_windows(counts):
    total = int(counts.sum())
    target = total / N_WIN_TOTAL
    wins = []
    g = 0
    rows_done = 0
    for w in range(N_WIN_TOTAL):
        g0 = g
        rows_w = 0
        while g < NUM_GRAPHS:
            c = int(counts[g])
            if rows_w + c > ROWS_PER_WIN or (g - g0) >= G_WIN:
                break
            if (w < N_WIN_TOTAL - 1 and rows_w > 0
                    and rows_done + rows_w + c > (w + 1) * target):
                remaining = total - (rows_done + rows_w)
                if remaining <= (N_WIN_TOTAL - w - 1) * ROWS_PER_WIN * 0.98:
                    break
            rows_w += c
            g += 1
        while g < NUM_GRAPHS and counts[g] == 0 and (g - g0) < G_WIN:
            g += 1
        rows_done += rows_w
        wins.append((g0, g))
    assert g == NUM_GRAPHS, f"window packing failed: {g}/{NUM_GRAPHS}"
    return wins


def _build_core_inputs(h, idx, counts, starts, wins, core, shared):
    f16 = _f16()
    f8 = _f8()
    h_pad = np.zeros((N_LOC, D), dtype=np.float32)
    slot = np.full(N_LOC, -1, dtype=np.int64)
    invc = np.zeros((P, WIN_PER_CORE), dtype=np.float32)
    gmap = []
    for lw in range(WIN_PER_CORE):
        g0, g1 = wins[core * WIN_PER_CORE + lw]
        r0, r1 = int(starts[g0]), int(starts[g1])
        n = r1 - r0
        h_pad[lw * ROWS_PER_WIN: lw * ROWS_PER_WIN + n] = h[r0:r1]
        slot[lw * ROWS_PER_WIN: lw * ROWS_PER_WIN + n] = \
            lw * G_WIN + (idx[r0:r1] - g0)
        for j, g in enumerate(range(g0, g1)):
            invc[j, lw] = 1.0 / max(int(counts[g]), 1)
            gmap.append((g, lw * G_WIN + j))
    # per-chunk one-hots; B variants have the row order reversed within each
    # chunk (SWInterleave fc reverses psum/h row order on odd fc layers)
    wsc = np.zeros((CHUNKS, P, P), dtype=np.float32)   # [c][row, g]
    for c in range(CHUNKS):
        w = c // CH_PER_WIN
        s = slot[c * P:(c + 1) * P]
        real = np.nonzero(s >= 0)[0]
        wsc[c][real, s[real] - w * G_WIN] = 1.0
    wsc_rev = wsc[:, ::-1, :]
    wga = np.transpose(wsc, (0, 2, 1))                 # [c][g, row]
    wga_rev = np.transpose(wsc_rev, (0, 2, 1))

    def flat(a, dt):
        return np.ascontiguousarray(
            np.transpose(a, (1, 0, 2)).reshape(P, CHUNKS * P)).astype(dt)

    # scatter pair layout: [p, pc, i, g] = wsc[2pc+i][p, g]
    def scpair(a):
        return np.ascontiguousarray(
            a.reshape(N_PAIRS, 2, P, P).transpose(2, 0, 1, 3)
            .reshape(P, N_PAIRS * 2 * P)).astype(f8)

    h3 = h_pad.reshape(CHUNKS, P, DBLK, P)
    h16t = np.ascontiguousarray(h3.transpose(3, 0, 2, 1).reshape(P, CHUNKS * D))
    Wsum0, cbias0 = shared["_wsum0"], shared["_cbias0"]
    ssum = np.zeros((G_LOC, D), dtype=np.float32)
    np.add.at(ssum, slot[slot >= 0], h_pad[slot >= 0])
    mean0 = ssum * invc.T.reshape(G_LOC, 1)
    x2w0 = (mean0 @ Wsum0 + cbias0).astype(f16)
    x2w0b = np.ascontiguousarray(
        x2w0.reshape(WIN_PER_CORE, G_WIN, D).transpose(1, 0, 2)
        .reshape(G_WIN, WIN_PER_CORE * D))
    in_map = {
        "h16t": h16t.astype(f16),
        "x2w0b": x2w0b,
        "wscA": scpair(wsc),
        "wscB": scpair(wsc_rev),
        "wgaA": flat(wga, f16),
        "wgaB": flat(wga_rev, f16),
        "invc": invc,
        **{k: v for k, v in shared.items() if not k.startswith("_")},
    }
    return in_map, gmap


def _prep_shared(Wfc, bfc, Wsum, bsum, Wf1, bf1, Wf2, bf2):
    f16 = _f16()
    f8 = _f8()
    # layer-0 fc weights fp16 (moving): [P, b, D] = Wfc[0][128b+p, :]
    wfc0 = np.ascontiguousarray(
        Wfc[0].reshape(DBLK, P, D).transpose(1, 0, 2)
        .reshape(P, DBLK * D)).astype(f16)
    # layers 1/2 fc weights fp8 DoubleRow moving pairs:
    # [p, li, b, i, :] = Wfc[l][256b + 2p + i, :]
    wfc8 = np.zeros((P, 2, 2, 2, D), dtype=np.float32)
    for li, l in enumerate((1, 2)):
        Wl = Wfc[l].astype(np.float32)
        for b in range(2):
            for i in range(2):
                wfc8[:, li, b, i, :] = Wl[256 * b + 2 * np.arange(P) + i]
    wfc8 = np.ascontiguousarray(wfc8.reshape(P, 2 * 2 * 2 * D)).astype(f8)
    Wsum_eff = Wsum.astype(np.float32).copy()
    for l in (1, 2):
        Wsum_eff[l] += Wfc[l].astype(np.float32) - _q8(Wfc[l])
    wsum = np.ascontiguousarray(
        Wsum_eff.reshape(N_LAYERS, DBLK, P, D).transpose(2, 0, 1, 3)
        .reshape(P, N_LAYERS * DBLK * D)).astype(f16)
    cbias = np.ascontiguousarray((bfc + bsum).reshape(1, N_LAYERS * D)).astype(f16)
    wf1 = np.ascontiguousarray(
        Wf1.reshape(DBLK, P, 2, D).transpose(1, 0, 2, 3)
        .reshape(P, DBLK * D2)).astype(f16)
    bf1w = np.ascontiguousarray(bf1.reshape(1, D2)).astype(f16)
    wf2 = np.ascontiguousarray(
        Wf2.reshape(D2BLK, P, NUM_TASKS).transpose(1, 0, 2)
        .reshape(P, D2BLK * NUM_TASKS)).astype(f16)
    bf2w = np.ascontiguousarray(bf2.reshape(1, NUM_TASKS)).astype(f16)
    return {
        "wfc0": wfc0, "wfc8": wfc8, "wsum": wsum, "cbias": cbias,
        "wf1": wf1, "bf1w": bf1w, "wf2": wf2, "bf2w": bf2w,
        "_wsum0": Wsum[0].astype(np.float32),
        "_cbias0": (bfc[0] + bsum[0]).astype(np.float32)[None, :],
    }


# -------------------------------------------------------------- bass program

def _build_program():
    from contextlib import ExitStack

    import concourse.mybir as mybir
    import concourse.tile as tile
    from concourse import bacc

    bf = mybir.dt.float16
    f8 = mybir.dt.float8e4
    f32 = mybir.dt.float32
    AF = mybir.ActivationFunctionType
    ALU = mybir.AluOpType
    DR = mybir.MatmulPerfMode.DoubleRow
    DRS = mybir.MatmulPerfMode.DoubleRowSwInterleave

    nc = bacc.Bacc("TRN2", debug=False, target_bir_lowering=False,
                   num_devices=N_CORES, dynamic_dma_scratch_size=2048)

    h16t_d = nc.dram_tensor("h16t", [P, CHUNKS * D], bf, kind="ExternalInput")
    x2w0_d = nc.dram_tensor("x2w0b", [G_WIN, WIN_PER_CORE * D], bf,
                            kind="ExternalInput")
    wscA_d = nc.dram_tensor("wscA", [P, N_PAIRS * 2 * P], f8, kind="ExternalInput")
    wscB_d = nc.dram_tensor("wscB", [P, N_PAIRS * 2 * P], f8, kind="ExternalInput")
    wgaA_d = nc.dram_tensor("wgaA", [P, CHUNKS * P], bf, kind="ExternalInput")
    wgaB_d = nc.dram_tensor("wgaB", [P, CHUNKS * P], bf, kind="ExternalInput")
    invc_d = nc.dram_tensor("invc", [P, WIN_PER_CORE], f32, kind="ExternalInput")
    wfc0_d = nc.dram_tensor("wfc0", [P, DBLK * D], bf, kind="ExternalInput")
    wfc8_d = nc.dram_tensor("wfc8", [P, 2 * 2 * 2 * D], f8, kind="ExternalInput")
    wsum_d = nc.dram_tensor("wsum", [P, N_LAYERS * DBLK * D], bf, kind="ExternalInput")
    cbias_d = nc.dram_tensor("cbias", [1, N_LAYERS * D], bf, kind="ExternalInput")
    wf1_d = nc.dram_tensor("wf1", [P, DBLK * D2BLK * P], bf, kind="ExternalInput")
    bf1_d = nc.dram_tensor("bf1w", [1, D2], bf, kind="ExternalInput")
    wf2_d = nc.dram_tensor("wf2", [P, D2BLK * NUM_TASKS], bf, kind="ExternalInput")
    bf2_d = nc.dram_tensor("bf2w", [1, NUM_TASKS], bf, kind="ExternalInput")
    out_d = nc.dram_tensor("out", [NUM_TASKS, G_LOC], f32, kind="ExternalOutput")

    with tile.TileContext(nc) as tc, ExitStack() as ctx:
        const = ctx.enter_context(tc.tile_pool(name="const", bufs=1))
        hpool = ctx.enter_context(tc.tile_pool(name="h", bufs=1))
        stream = ctx.enter_context(tc.tile_pool(name="stream", bufs=6))
        work = ctx.enter_context(tc.tile_pool(name="work", bufs=2))
        x2pool = ctx.enter_context(tc.tile_pool(name="x2", bufs=1))
        psum = ctx.enter_context(tc.tile_pool(name="psum", bufs=1, space="PSUM"))
        psx1 = ctx.enter_context(tc.tile_pool(name="psx1", bufs=6, space="PSUM"))

        ones = const.tile([1, P], bf, tag="ones")
        nc.vector.memset(ones[:], 1.0)
        x2w0_t = []
        for w in range(WIN_PER_CORE):
            t0w = x2pool.tile([P, D], bf, tag=f"x2w{w}", name=f"x2w0_{w}")
            nc.sync.dma_start(t0w[:], x2w0_d[:, w * D:(w + 1) * D])
            x2w0_t.append(t0w)
        WCH = CH_PER_WIN * P
        wgaA_w = [const.tile([P, WCH], bf, tag=f"wgaA{k}", name=f"wgaA{k}")
                  for k in range(WIN_PER_CORE)]
        wgaB_w = [const.tile([P, WCH], bf, tag=f"wgaB{k}", name=f"wgaB{k}")
                  for k in range(WIN_PER_CORE)]
        wscA_w = [const.tile([P, WCH], f8, tag=f"wscA{k}", name=f"wscA{k}")
                  for k in range(WIN_PER_CORE)]
        wscB_w = [const.tile([P, WCH], f8, tag=f"wscB{k}", name=f"wscB{k}")
                  for k in range(WIN_PER_CORE)]
        wfc0_t = const.tile([P, DBLK * D], bf, tag="wfc0")
        nc.sync.dma_start(wfc0_t[:], wfc0_d[:, :])
        invc_t = const.tile([P, WIN_PER_CORE], f32, tag="invc")
        nc.sync.dma_start(invc_t[:], invc_d[:, :])
        # fp8 row-major h, chunk-pair tiles [128, 2, 512]
        h_t = [hpool.tile([P, 2 * D], f8, tag=f"h{pc}", name=f"h{pc}")
               for pc in range(N_PAIRS)]

        def h_chunk(c):
            return h_t[c // 2][:, (c % 2) * D:(c % 2 + 1) * D]

        # h0T streamed in 4-chunk tiles (few large DMAs keep the sync queue
        # short so the layer-1/2 transposes are not stuck behind it)
        hT0 = []
        for q in range(CHUNKS // 4):
            ht = stream.tile([P, 4 * D], bf, tag="hT", name=f"hT0_{q}", bufs=3)
            nc.sync.dma_start(ht[:], h16t_d[:, q * 4 * D:(q + 1) * 4 * D])
            hT0.append(ht)
        # one-hots + weights on the ACT HWDGE queue (A-parity first: needed
        # from the very first layer-0 chunk)
        for k in range(WIN_PER_CORE):
            nc.scalar.dma_start(wgaA_w[k][:], wgaA_d[:, k * WCH:(k + 1) * WCH])
            nc.scalar.dma_start(wscA_w[k][:], wscA_d[:, k * WCH:(k + 1) * WCH])
        wfc8_t = const.tile([P, 2 * 2 * 2 * D], f8, tag="wfc8")
        nc.scalar.dma_start(wfc8_t[:], wfc8_d[:, :])
        for k in range(WIN_PER_CORE):
            nc.scalar.dma_start(wgaB_w[k][:], wgaB_d[:, k * WCH:(k + 1) * WCH])
            nc.scalar.dma_start(wscB_w[k][:], wscB_d[:, k * WCH:(k + 1) * WCH])
        wsumb = const.tile([P, N_LAYERS * DBLK * D], bf, tag="wsumb")
        nc.scalar.dma_start(wsumb[:], wsum_d[:, :])
        cbiasb = const.tile([1, N_LAYERS * D], bf, tag="cbiasb")
        nc.scalar.dma_start(cbiasb[:], cbias_d[:, :])
        wf1b = const.tile([P, DBLK * D2BLK * P], bf, tag="wf1b")
        nc.scalar.dma_start(wf1b[:], wf1_d[:, :])
        bf1_t = const.tile([1, D2], bf, tag="bf1")
        nc.scalar.dma_start(bf1_t[:], bf1_d[:, :])
        wf2b = const.tile([P, D2BLK * NUM_TASKS], bf, tag="wf2b")
        nc.scalar.dma_start(wf2b[:], wf2_d[:, :])
        bf2_t = const.tile([1, NUM_TASKS], bf, tag="bf2")
        nc.scalar.dma_start(bf2_t[:], bf2_d[:, :])

        def wfc0_s(b):
            return wfc0_t[:, b * D:(b + 1) * D]

        def wfc8_s(layer, b):
            off = (((layer - 1) * 2 + b) * 2) * D
            return wfc8_t[:, off:off + 2 * D].rearrange("p (i d) -> p i d", i=2)

        def wsum_s(layer, b):
            return wsumb[:, (layer * DBLK + b) * D:(layer * DBLK + b + 1) * D]

        def cbias_s(layer):
            return cbiasb[:, layer * D:(layer + 1) * D]

        def wf1_h(b, half):
            i = b * 2 + half
            return wf1b[:, i * D:(i + 1) * D]

        def wf2_s(q):
            return wf2b[:, q * NUM_TASKS:(q + 1) * NUM_TASKS]

        # parity: h order is natural after layers 0/2, reversed after layer 1
        def wga_c(c, layer):
            tiles = wgaB_w if layer == 1 else wgaA_w
            return tiles[c // CH_PER_WIN][:, (c % CH_PER_WIN) * P:
                                          (c % CH_PER_WIN + 1) * P]

        def wsc_pair(pc, layer):
            # scatter of h produced by `layer`: reversed iff layer == 1
            tiles = wscB_w if layer == 1 else wscA_w
            j = (2 * pc) % CH_PER_WIN
            return tiles[(2 * pc) // CH_PER_WIN][:, j * P:(j + 2) * P] \
                .rearrange("p (i g) -> p i g", i=2)

        hT_next = hT0

        def transpose_pair(pc, tag, name, eng=None):
            """chunk-pair fp8 -> SWInterleave lhsT via u16-pair DMA transpose."""
            ht = stream.tile([P, 2 * D], f8, tag=tag, name=name, bufs=12)
            (eng or nc.sync).dma_start(
                ht[:].bitcast(bf).rearrange("p (k r) -> p k r", k=4),
                h_t[pc][:].bitcast(bf), transpose=True)
            return ht

        def x2_window(meanT, w, layer):
            ps = psum.tile([P, D], f32, tag="x2")
            for b in range(DBLK):
                nc.tensor.matmul(ps[:], lhsT=meanT[:, b * P:(b + 1) * P],
                                 rhs=wsum_s(layer, b),
                                 start=(b == 0), stop=False)
            nc.tensor.matmul(ps[:], lhsT=ones[:, :P], rhs=cbias_s(layer),
                             start=False, stop=True)
            x2w = x2pool.tile([P, D], bf, tag=f"x2w{w}", name=f"x2w{layer}_{w}")
            nc.scalar.activation(x2w[:], ps[:], AF.Copy)
            return x2w[:]

        x2ws = {w: x2w0_t[w][:] for w in range(WIN_PER_CORE)}

        out_sb = const.tile([NUM_TASKS, G_LOC], f32, tag="out")

        head_pend = {}

        def head_t(w, meanT):
            t = work.tile([P, D2], bf, tag="tT", bufs=1, name=f"t{w}")
            tTh = []
            for half in range(2):
                ps = psx1.tile([P, D], f32, tag="x1", name=f"hps{w}_{half}")
                for b in range(DBLK):
                    nc.tensor.matmul(ps[:], lhsT=meanT[:, b * P:(b + 1) * P],
                                     rhs=wf1_h(b, half),
                                     start=(b == 0), stop=False)
                nc.tensor.matmul(ps[:], lhsT=ones[:, :P],
                                 rhs=bf1_t[:, half * D:(half + 1) * D],
                                 start=False, stop=True)
                nc.scalar.activation(t[:, half * D:(half + 1) * D],
                                     ps[:], AF.Relu)
                th = work.tile([P, D], bf, tag=f"tTh{half}", bufs=2,
                               name=f"tTh{w}_{half}")
                nc.scalar.dma_start(
                    th[:].rearrange("p (b r) -> p b r", b=DBLK),
                    t[:, half * D:(half + 1) * D], transpose=True)
                tTh.append(th)
            head_pend[w] = tTh

        def head_out(w):
            tTh = head_pend.pop(w)
            pso = psum.tile([NUM_TASKS, P], f32, tag="x2", name=f"pso{w}")
            for q in range(D2BLK):
                nc.tensor.matmul(pso[:], lhsT=wf2_s(q),
                                 rhs=tTh[q // 4][:, (q % 4) * P:
                                                 (q % 4 + 1) * P],
                                 start=(q == 0), stop=False)
            nc.tensor.matmul(pso[:], lhsT=bf2_t[:], rhs=ones[:, :P],
                             start=False, stop=True)
            nc.vector.tensor_copy(out_sb[:, w * P:(w + 1) * P], pso[:])

        LAG = 4
        for layer in range(N_LAYERS):
            hTts = hT_next
            if layer > 0:
                for pc in range(len(hTts), N_PAIRS):
                    hTts.append(
                        transpose_pair(pc, "hT8", f"hT{layer}_{pc}"))
            hT_next = []
            nxt_x2ws = {}
            sc_state = {}
            pend = {}

            def emit_pair_scatter(pc, layer=layer, sc_state=sc_state,
                                  pend=pend):
                i = pc % (CH_PER_WIN // 2)
                w = pc // (CH_PER_WIN // 2)
                if i == 0:
                    sc_state["ps"] = psum.tile([P, D], f32, tag="sc",
                                               name=f"sc{layer}_{w}")
                nc.tensor.matmul(
                    sc_state["ps"][:], lhsT=wsc_pair(pc, layer),
                    rhs=h_t[pc][:].rearrange("p (i d) -> p i d", i=2),
                    start=(i == 0), stop=(i == CH_PER_WIN // 2 - 1),
                    perf_mode=DR)
                if i == CH_PER_WIN // 2 - 1:
                    mean = work.tile([P, D], bf, tag="mean", bufs=1,
                                     name=f"mean{layer}_{w}")
                    nc.vector.tensor_scalar(mean[:], sc_state["ps"][:],
                                            invc_t[:, w:w + 1], None,
                                            op0=ALU.mult)
                    meanT = work.tile([P, D], bf, tag="meanT", bufs=1,
                                      name=f"meanT{layer}_{w}")
                    nc.scalar.dma_start(
                        meanT[:].rearrange("p (b r) -> p b r", b=DBLK),
                        mean[:], transpose=True)
                    pend[w] = meanT

            def emit_window_tail(w, layer=layer, nxt_x2ws=nxt_x2ws,
                                 pend=pend):
                meanT = pend.pop(w)
                if layer < N_LAYERS - 1:
                    nxt_x2ws[w] = x2_window(meanT, w, layer + 1)
                else:
                    head_t(w, meanT)

            for c in range(CHUNKS):
                w = c // CH_PER_WIN
                ps = psx1.tile([P, D], f32, tag="x1")
                if layer == 0:
                    hTt = hTts[c // 4]
                    co = (c % 4) * D
                    for b in range(DBLK):
                        nc.tensor.matmul(ps[:],
                                         lhsT=hTt[:, co + b * P:
                                                  co + (b + 1) * P],
                                         rhs=wfc0_s(b),
                                         start=(b == 0), stop=False)
                else:
                    hTt = hTts[c // 2]
                    for b in range(2):
                        lhsT = hTt[:, (2 * (c % 2) + b) * 2 * P:
                                   (2 * (c % 2) + b + 1) * 2 * P] \
                            .rearrange("p (m i) -> p m i", i=2)
                        nc.tensor.matmul(ps[:], lhsT=lhsT,
                                         rhs=wfc8_s(layer, b),
                                         start=(b == 0), stop=False,
                                         perf_mode=DRS)
                nc.tensor.matmul(ps[:], lhsT=wga_c(c, layer), rhs=x2ws[w],
                                 start=False, stop=True)
                # ELU: h = max(z, min(exp(z), 1) - 1)
                e = work.tile([P, D], bf, tag="e", bufs=3)
                nc.scalar.activation(e[:], ps[:], AF.Exp)
                me = work.tile([P, D], bf, tag="me", bufs=3)
                nc.vector.tensor_scalar(me[:], e[:], 1.0, -1.0,
                                        op0=ALU.min, op1=ALU.add)
                nc.vector.tensor_tensor(h_chunk(c), ps[:], me[:],
                                        op=ALU.max)
                if layer < N_LAYERS - 1 and c % 2 == 1 and c < 2 * PREFETCH:
                    hT_next.append(
                        transpose_pair(c // 2, "hT8p", f"hTp{layer}_{c // 2}",
                                       eng=nc.scalar))
                if c >= LAG and (c - LAG) % 2 == 1:
                    emit_pair_scatter((c - LAG) // 2)
                cw = (c - LAG - 8) // CH_PER_WIN
                if c >= LAG + 8 and (c - LAG - 8) % CH_PER_WIN == CH_PER_WIN - 1:
                    emit_window_tail(cw)
                cw2 = (c - LAG - 14) // CH_PER_WIN
                if (c >= LAG + 14
                        and (c - LAG - 14) % CH_PER_WIN == CH_PER_WIN - 1
                        and cw2 in head_pend):
                    head_out(cw2)
            for pc in range((CHUNKS - LAG) // 2, N_PAIRS):
                emit_pair_scatter(pc)
            for w in sorted(pend):
                emit_window_tail(w)
            for w in sorted(head_pend):
                head_out(w)
            x2ws = nxt_x2ws

        nc.sync.dma_start(out_d[:, :], out_sb[:])

    nc.compile()
    return nc


# ------------------------------------------------------------------- kernel

def kernel(**inputs):
    h = np.asarray(inputs["h_subgraph"], dtype=np.float32)
    idx = np.asarray(inputs["subgraph_idx_batch"]).astype(np.int64)
    if not np.all(idx[:-1] <= idx[1:]):
        order = np.argsort(idx, kind="stable")
        h, idx = h[order], idx[order]

    counts = np.bincount(idx, minlength=NUM_GRAPHS)
    starts = np.concatenate([[0], np.cumsum(counts)])
    wins = _pack_windows(counts)
    shared = _prep_shared(
        np.asarray(inputs["Wfc"], np.float32), np.asarray(inputs["bfc"], np.float32),
        np.asarray(inputs["Wsum"], np.float32), np.asarray(inputs["bsum"], np.float32),
        np.asarray(inputs["Wf1"], np.float32), np.asarray(inputs["bf1"], np.float32),
        np.asarray(inputs["Wf2"], np.float32), np.asarray(inputs["bf2"], np.float32),
    )

    in_maps = []
    gmaps = []
    for core in range(N_CORES):
        m, gm = _build_core_inputs(h, idx, counts, starts, wins, core, shared)
        in_maps.append(m)
        gmaps.append(gm)

    _cached["in_maps"] = in_maps
    if "nc" not in _cached:
        _cached["nc"] = _build_program()
    nc = _cached["nc"]

    from concourse import bass_utils
    res = bass_utils.run_bass_kernel_spmd(
        nc, in_maps, core_ids=list(range(N_CORES)))

    out = np.zeros((NUM_GRAPHS, NUM_TASKS), dtype=np.float32)
    for core in range(N_CORES):
        o = res.results[core]["out"]           # [10, 640]
        for g, s in gmaps[core]:
            out[g] = o[:, s]
    return out
